# revision 40
# baseline (speedup 1.0000x reference)
"""BitNet attention layer (quantized QKV + attention + quantized dense + LN)
as a Bass/Tile SPMD kernel for 8 Trainium2 NeuronCores.

Sharding: core c = 2*b + g handles batch b (of 4) and head-group g (of 2,
8 heads each).  The host permutes the token axis per core so each core's
own 1024 tokens sit in columns 0:1024 (making the program g-independent):
QKV projection + attention are fully local; after each partner-half
attention chunk finishes, it is shipped to the paired core via a pair
AllGather hidden under the remaining attention matmuls.  The dense output
projection then runs with the FULL 2048-deep contraction on each core's
own token half (host also permutes W_dense rows to [own heads, partner
heads]) -- no output ReduceScatter and no serial tail: residual+layernorm
stream right behind the dense matmuls.  The partner AllGather slot is
selected with a host-fed {0,1} mask pair folded into the ctx quantize.

The softmax denominator (a partition-axis reduction) is computed with
ones-column matmuls packed 4-to-a-pass into distinct PE col-groups via
tile_position, costing ~1/4 of a full matmul stream; the 4 partial rows
are combined by DVE reads of the PSUM rows.  Row reciprocals use the fast
custom-DVE approximation (~18 bits).

Numerics: activations are round()ed to ints in [-127,127] and weights to
{-1,0,1} ({-2,0,2} for the sign-quantized W_v/W_d, 0.5 folded into the
dequant scales); all exactly representable in f16, and f32 PSUM
accumulation of <=2048 such products is exact.  With zero biases (the
benchmark instance) projection outputs stay RAW integer sums and the
dequant scales fold downstream.  Magic-round constant 1536 keeps q+magic
inside the f16 ulp=1 binade for both signs, so rounding matches
jnp.round exactly.
"""

import math
import sys

import numpy as np

sys.path.insert(0, "/opt/trn_rl_repo")

import concourse.bacc as bacc
import concourse.bass as bass
import concourse.bass_isa as bass_isa
import concourse.mybir as mybir
import concourse.tile as tile

F32 = mybir.dt.float32
F16 = mybir.dt.float16
BF16 = mybir.dt.bfloat16
AF = mybir.ActivationFunctionType
OP = mybir.AluOpType

P = 128
H = 2048
S = 2048
B = 4
NH = 16
HD = 128
NCORES = 8
TOK = S
HB = H // P            # 16 h blocks
NHC = NH // 2          # 8 heads per core
HALF = TOK // 2        # 1024 tokens kept per core
MG = 1536.0            # f16 magic: q+MG stays in [1024,2048) => ulp 1
INV_SQD = 1.0 / math.sqrt(HD)
LN_EPS = 1e-5
PAIRS = [[0, 1], [2, 3], [4, 5], [6, 7]]
ALL8 = [list(range(NCORES))]


def _chunks(count, width, base=0):
    return [slice(base + i * width, base + (i + 1) * width) for i in range(count)]


def build_program(use_mask: bool, qk_bias_zero: bool, v_bias_zero: bool,
                  d_bias_zero: bool, ln_trivial: bool):
    nc = bacc.Bacc("TRN2", target_bir_lowering=False, debug=False,
                   enable_asserts=False, num_devices=NCORES)

    # ---- I/O (token axis per-core permuted: own half first) --------------
    xt = nc.dram_tensor("xt", [H, TOK], F16, kind="ExternalInput")
    xr = nc.dram_tensor("xr", [HALF, H], F32, kind="ExternalInput")
    # W_qk^T column-slab-tiled: [ob, p, kb*128+c] = W^T[kb*128+p, ob*128+c]
    wqkt = nc.dram_tensor("wqkt", [16, P, 2048], F16, kind="ExternalInput")
    wvt = nc.dram_tensor("wvt", [H, 1024], F16, kind="ExternalInput")
    bqk = nc.dram_tensor("bqk", [P, 16], F32, kind="ExternalInput")
    bv = nc.dram_tensor("bv", [1, 1024], F32, kind="ExternalInput")
    # FULL W_dense^T, rows permuted to [own head half, partner head half]
    wdt = nc.dram_tensor("wdt", [H, H], F16, kind="ExternalInput")
    bdh = nc.dram_tensor("bdh", [1, H], F32, kind="ExternalInput")
    maskt = nc.dram_tensor("maskt", [P, HB], F32, kind="ExternalInput")
    csel = nc.dram_tensor("csel", [1, 16], F32, kind="ExternalInput")
    pmsk = nc.dram_tensor("pmsk", [1, 2], F32, kind="ExternalInput")
    lnw = nc.dram_tensor("lnw", [1, H], F32, kind="ExternalInput")
    lnb = nc.dram_tensor("lnb", [1, H], F32, kind="ExternalInput")
    out = nc.dram_tensor("out", [HALF, H], F32, kind="ExternalOutput")

    # ---- DRAM scratch ----------------------------------------------------
    qkt_d = nc.dram_tensor("qkt_d", [16, P, TOK], F16)
    vt_d = nc.dram_tensor("vt_d", [16, P, 1024], BF16)
    exch_i = nc.dram_tensor("exch_i", [NHC, P, HALF], F16)
    exch_o = nc.dram_tensor("exch_o", [NHC, 2, P, HALF], F16)
    c_add_i = nc.dram_tensor("c_add_i", [P, 1], F32)
    c_add_o = nc.dram_tensor("c_add_o", [P, 1], F32)
    c_mx_i = nc.dram_tensor("c_mx_i", [P, 1], F32)
    c_mx_o = nc.dram_tensor("c_mx_o", [P, 1], F32)
    c_mc_i = nc.dram_tensor("c_mc_i", [P, 1], F32)
    c_mc_o = nc.dram_tensor("c_mc_o", [P, 1], F32)
    c_wu_i = nc.dram_tensor("c_wu_i", [1, 16], F32)
    c_wu_o = nc.dram_tensor("c_wu_o", [1, 16], F32)

    with tile.TileContext(nc) as tc:
        _emit(tc, locals(), use_mask, qk_bias_zero, v_bias_zero,
              d_bias_zero, ln_trivial)

    nc.compile()
    return nc


def _emit(tc, T, use_mask, qk_bias_zero, v_bias_zero, d_bias_zero,
          ln_trivial):
    nc = tc.nc
    xt, xr, wqkt, wvt, bqk, bv, wdt, bdh = (T["xt"], T["xr"], T["wqkt"],
                                            T["wvt"], T["bqk"], T["bv"],
                                            T["wdt"], T["bdh"])
    maskt, lnw, lnb, out = T["maskt"], T["lnw"], T["lnb"], T["out"]
    csel, pmsk = T["csel"], T["pmsk"]
    qkt_d, vt_d = T["qkt_d"], T["vt_d"]
    exch_i, exch_o = T["exch_i"], T["exch_o"]
    c_add_i, c_add_o = T["c_add_i"], T["c_add_o"]
    c_mx_i, c_mx_o = T["c_mx_i"], T["c_mx_o"]
    c_mc_i, c_mc_o = T["c_mc_i"], T["c_mc_o"]
    c_wu_i, c_wu_o = T["c_wu_i"], T["c_wu_o"]

    from contextlib import ExitStack

    est = ExitStack()
    with est:
        smalls = est.enter_context(tc.tile_pool(name="smalls", bufs=1))
        stream2 = est.enter_context(tc.tile_pool(name="stream2", bufs=2))
        red = est.enter_context(tc.tile_pool(name="red", bufs=4))

        def sc_tile(name, shape=(1, 1)):
            return smalls.tile(list(shape), F32, tag=name, name=name)

        def bcast(name, src):
            b = sc_tile(name, (P, 1))
            nc.gpsimd.partition_broadcast(b[:], src[:])
            return b

        ones_col = smalls.tile([P, 1], BF16, tag="ones_col")
        nc.vector.memset(ones_col[:], 1.0)
        mgb = smalls.tile([P, 1], F32, tag="mgb")
        nc.vector.memset(mgb[:], MG)
        csb = smalls.tile([1, 16], F32, tag="csb")
        nc.sync.dma_start(csb[:], csel[:, :])
        pms = smalls.tile([1, 2], F32, tag="pms")
        nc.sync.dma_start(pms[:], pmsk[:, :])
        m0 = sc_tile("m0")
        nc.vector.tensor_copy(m0[:], pms[0:1, 0:1])
        m1 = sc_tile("m1")
        nc.vector.tensor_copy(m1[:], pms[0:1, 1:2])
        m0_b = bcast("m0_b", m0)
        m1_b = bcast("m1_b", m1)
        bqk_sb = None
        if not qk_bias_zero:
            bqk_sb = smalls.tile([P, 16], F32, tag="bqk_sb")
            nc.sync.dma_start(bqk_sb[:], bqk[:, :])
        mask_sb = None
        if use_mask:
            mask_sb = smalls.tile([P, HB], F32, tag="mask_sb")
            nc.sync.dma_start(mask_sb[:], maskt[:, :])

        # ============ Stage 0a: load x, max|x| -> AR_x =====================
        s1es = ExitStack()
        xq_pool = s1es.enter_context(tc.tile_pool(name="xq", bufs=HB))
        wda_es = ExitStack()
        wda_pool = wda_es.enter_context(tc.tile_pool(name="wda", bufs=2))
        xq = []
        xmax = sc_tile("xmax", (P, 1))
        for t in range(HB):
            xf = xq_pool.tile([P, TOK], F16, tag="xq")
            nc.sync.dma_start(xf[:], xt[t * P:(t + 1) * P, :])
            xq.append(xf)
            r = red.tile([P, 1], F32, tag="xred")
            nc.vector.tensor_reduce(r[:], xf[:], axis=mybir.AxisListType.X,
                                    op=OP.max, apply_absolute_value=True)
            if t == 0:
                nc.vector.tensor_copy(xmax[:], r[:])
            else:
                nc.vector.tensor_tensor(xmax[:], xmax[:], r[:], OP.max)
        nc.sync.dma_start(c_mx_i[:, :], xmax[:])
        nc.gpsimd.collective_compute(
            "AllReduce", OP.max, replica_groups=ALL8,
            ins=[c_mx_i[:, :].opt()], outs=[c_mx_o[:, :].opt()])

        # ============ Stage 0b: load W_qk + W_v, |W| abs -> AR_A ===========
        accA = sc_tile("accA", (P, 1))
        ps0 = ExitStack()
        ps0_pool = ps0.enter_context(tc.tile_pool(name="ps0", bufs=1,
                                                  space="PSUM"))
        absdump = ps0_pool.tile([P, 2048], F32, tag="absdump")

        wv_es = ExitStack()
        wv_pool = wv_es.enter_context(tc.tile_pool(name="wv16", bufs=HB))
        wq_es = ExitStack()
        wq_pool = wq_es.enter_context(tc.tile_pool(name="wq16", bufs=HB))

        wq16 = []
        for ob in range(HB):
            w16 = wq_pool.tile([P, 2048], F16, tag="wq16")
            nc.sync.dma_start(w16[:], wqkt[ob, :, :])
            r = red.tile([P, 1], F32, tag="wred")
            if ob < 8:
                nc.scalar.activation(absdump[:], w16[:], AF.Abs,
                                     accum_out=r[:])
            else:
                nc.vector.tensor_reduce(r[:], w16[:],
                                        axis=mybir.AxisListType.X,
                                        op=OP.add, apply_absolute_value=True)
            if ob == 0:
                nc.vector.tensor_copy(accA[:], r[:])
            else:
                nc.vector.tensor_tensor(accA[:], accA[:], r[:], OP.add)
            wq16.append(w16)

        wv16 = []
        for t in range(HB):
            wf = wv_pool.tile([P, 1024], F16, tag="wv16")
            nc.sync.dma_start(wf[:], wvt[t * P:(t + 1) * P, :])
            r = red.tile([P, 1], F32, tag="wred")
            nc.vector.tensor_reduce(r[:], wf[:], axis=mybir.AxisListType.X,
                                    op=OP.add, apply_absolute_value=True)
            nc.vector.tensor_tensor(accA[:], accA[:], r[:], OP.add)
            wv16.append(wf)

        nc.sync.dma_start(c_add_i[:, :], accA[:])
        nc.gpsimd.collective_compute(
            "AllReduce", OP.add, replica_groups=ALL8,
            ins=[c_add_i[:, :].opt()], outs=[c_add_o[:, :].opt()])

        ps0.close()

        # ============ scales (x first: xq is on the critical path) =========
        xm = sc_tile("xm", (P, 1))
        nc.sync.dma_start(xm[:], c_mx_o[:, :])
        xmaxr = sc_tile("xmaxr", (P, 1))
        nc.gpsimd.partition_all_reduce(xmaxr[:], xm[:], channels=P,
                                       reduce_op=bass_isa.ReduceOp.max)
        xm1 = sc_tile("xm1", (P, 1))
        nc.vector.tensor_scalar(xm1[:], xmaxr[:], 1e-8, None, OP.add)
        rxm = sc_tile("rxm", (P, 1))
        nc.vector.reciprocal(rxm[:], xm1[:])
        sx_b = sc_tile("sx_b", (P, 1))
        nc.vector.tensor_scalar(sx_b[:], rxm[:], 127.0, None, OP.mult)

        # quantize x IN PLACE: round(x*sx) -> f16 ints (Act/DVE split)
        for kb in range(HB):
            t1 = stream2.tile([P, TOK], F16, tag="t2048")
            if kb < 8:
                nc.scalar.activation(t1[:], xq[kb][:], AF.Identity,
                                     bias=mgb[:], scale=sx_b[:])
            else:
                nc.vector.tensor_scalar(t1[:], xq[kb][:], sx_b[:], MG,
                                        OP.mult, OP.add)
            nc.vector.tensor_scalar(xq[kb][:], t1[:], MG, None, OP.subtract)

        # gamma_qkv = sum|W_qkv|/(3H*H)+1e-5 (all-8 add = 4x full sum)
        wsA = sc_tile("wsA", (P, 1))
        nc.sync.dma_start(wsA[:], c_add_o[:, :])
        accAr = sc_tile("accAr", (P, 1))
        nc.gpsimd.partition_all_reduce(accAr[:], wsA[:], channels=P,
                                       reduce_op=bass_isa.ReduceOp.add)
        gq = sc_tile("gq", (P, 1))
        nc.vector.tensor_scalar(gq[:], accAr[:],
                                1.0 / (4 * 3 * H * H), 1e-5, OP.mult, OP.add)
        igq_b = sc_tile("igq_b", (P, 1))
        nc.vector.reciprocal(igq_b[:], gq[:])

        # remaining stage-1/2 scales
        al_t = sc_tile("al_t", (P, 1))
        nc.vector.tensor_tensor(al_t[:], gq[:], xm1[:], OP.mult)
        alpha_b = sc_tile("alpha_b", (P, 1))
        nc.vector.tensor_scalar(alpha_b[:], al_t[:], 1.0 / 127.0, None,
                                OP.mult)
        a2_b = sc_tile("a2_b", (P, 1))
        nc.vector.tensor_tensor(a2_b[:], alpha_b[:], alpha_b[:], OP.mult)
        nc.vector.tensor_scalar(a2_b[:], a2_b[:], INV_SQD, None, OP.mult)
        # sign-route (W_v) gives {-2,0,2}; alpv carries the 0.5
        alpv_b = sc_tile("alpv_b", (P, 1))
        nc.vector.tensor_scalar(alpv_b[:], alpha_b[:], 0.5, None, OP.mult)
        ntq_b = sc_tile("ntq_b", (P, 1))
        nc.vector.tensor_scalar(ntq_b[:], gq[:], -0.5, None, OP.mult)
        ptq_b = sc_tile("ptq_b", (P, 1))
        nc.vector.tensor_scalar(ptq_b[:], gq[:], 0.5, None, OP.mult)

        bvb = None
        if not v_bias_zero:
            bv_sb = smalls.tile([1, 1024], F32, tag="bv_sb")
            nc.sync.dma_start(bv_sb[:], bv[:, :])
            bvb = smalls.tile([P, 1024], F32, tag="bvb")
            nc.gpsimd.partition_broadcast(bvb[:], bv_sb[:])

        # ============ W_d abs pass (streamed; overlaps QK) =================
        # wda pool stays open through stage 1 so its addresses are not
        # recycled by the QK eviction pool (address reuse would serialize
        # the evictions behind this DMA-paced pass).
        accD = sc_tile("accD", (P, 1))

        def wv_sign_unit(t, sgv_pool):
            s1 = sgv_pool.tile([P, 1024], F16, tag="sg16")
            nc.scalar.activation(s1[:], wv16[t][:], AF.Sign, bias=ntq_b[:])
            s2 = sgv_pool.tile([P, 1024], F16, tag="sg16")
            nc.scalar.activation(s2[:], wv16[t][:], AF.Sign, bias=ptq_b[:])
            nc.vector.tensor_tensor(wv16[t][:], s1[:], s2[:], OP.add)

        def wda_abs_unit(t):
            wt = wda_pool.tile([P, H], F16, tag="wda", name="wda%d" % t)
            nc.sync.dma_start(wt[:], wdt[t * P:(t + 1) * P, :])
            r = red.tile([P, 1], F32, tag="dred")
            nc.scalar.activation(wt[:], wt[:], AF.Abs, accum_out=r[:])
            if t == 0:
                nc.gpsimd.tensor_copy(accD[:], r[:])
            else:
                nc.gpsimd.tensor_tensor(accD[:], accD[:], r[:], OP.add)
        # ============ Stage 1: QK projection (lazy W quantize) =============
        with tc.tile_pool(name="s1ev", bufs=2) as ev_pool, \
             tc.tile_pool(name="sgv", bufs=3) as sgv_pool, \
             tc.tile_pool(name="ps1", bufs=2, space="PSUM") as ps1:
            for ob in range(16):
                wv_sign_unit(ob, sgv_pool)
                wda_abs_unit(ob)
                # ternary round(w/gq), clip [-1,1], in place (DVE magic)
                t1 = stream2.tile([P, 2048], F16, tag="t2048")
                nc.vector.tensor_scalar(t1[:], wq16[ob][:], igq_b[:], MG,
                                        OP.mult, OP.add)
                t2 = stream2.tile([P, 2048], F16, tag="t2048")
                nc.vector.tensor_scalar(t2[:], t1[:], MG, 1.0,
                                        OP.subtract, OP.min)
                nc.vector.tensor_scalar(wq16[ob][:], t2[:], -1.0, None,
                                        OP.max)
                psum = ps1.tile([P, TOK], F32, tag="ps")
                for kb in range(HB):
                    for sl in _chunks(4, 512):
                        nc.tensor.matmul(psum[:, sl],
                                         lhsT=wq16[ob][:, kb * P:(kb + 1) * P],
                                         rhs=xq[kb][:, sl],
                                         start=(kb == 0), stop=(kb == HB - 1))
                ev = ev_pool.tile([P, TOK], F16, tag="ev")
                if qk_bias_zero:
                    nc.scalar.activation(ev[:], psum[:], AF.Identity,
                                         bias=0.0, scale=1.0)
                else:
                    nc.scalar.activation(ev[:], psum[:], AF.Identity,
                                         bias=bqk_sb[:, ob:ob + 1],
                                         scale=alpha_b[:])
                nc.sync.dma_start(qkt_d[ob, :, :], ev[:])
        wq_es.close()

        with tc.tile_pool(name="evv", bufs=3) as evv_pool, \
             tc.tile_pool(name="ps1v", bufs=2, space="PSUM") as ps1v:
            for tb in range(HB):
                psum = ps1v.tile([P, 1024], F32, tag="ps")
                for kb in range(HB):
                    for sl in _chunks(2, 512):
                        nc.tensor.matmul(
                            psum[:, sl],
                            lhsT=xq[kb][:, tb * P:(tb + 1) * P],
                            rhs=wv16[kb][:, sl],
                            start=(kb == 0), stop=(kb == HB - 1))
                v = evv_pool.tile([P, 1024], BF16, tag="vt")
                if v_bias_zero:
                    nc.scalar.activation(v[:], psum[:], AF.Identity,
                                         bias=0.0, scale=1.0)
                else:
                    nc.vector.scalar_tensor_tensor(v[:], psum[:], alpv_b[:],
                                                   bvb[:], OP.mult, OP.add)
                nc.sync.dma_start(vt_d[tb, :, :], v[:])
        wv_es.close()
        wda_es.close()
        s1es.close()

        accDr = sc_tile("accDr", (P, 1))
        nc.gpsimd.partition_all_reduce(accDr[:], accD[:], channels=P,
                                       reduce_op=bass_isa.ReduceOp.add)
        gd = sc_tile("gd", (P, 1))
        nc.vector.tensor_scalar(gd[:], accDr[:],
                                1.0 / (H * H), 1e-5, OP.mult, OP.add)
        igd_b = sc_tile("igd_b", (P, 1))
        nc.vector.reciprocal(igd_b[:], gd[:])

        # ============ W_d load (quantize deferred into attention) ==========
        wd_es = ExitStack()
        wd_pool = wd_es.enter_context(tc.tile_pool(name="wd_sb", bufs=HB))
        wd_sb = []
        for t in range(HB):
            w = wd_pool.tile([P, H], F16, tag="wd_sb", name="wd%d" % t)
            # software-DGE queue: keeps this 8MB burst off the HWDGE
            # queues that carry the latency-critical qt/kt/vh loads
            nc.gpsimd.dma_start(w[:], wdt[t * P:(t + 1) * P, :])
            wd_sb.append(w)
        wd_jobs = list(range(HB))

        def quantize_wd_tile():
            # one W_d ternary round(w/gd) per attention chunk, on DVE
            if not wd_jobs:
                return
            t = wd_jobs.pop(0)
            q1 = stream2.tile([P, H], F16, tag="t2048")
            nc.vector.tensor_scalar(q1[:], wd_sb[t][:], igd_b[:], MG,
                                    OP.mult, OP.add)
            q2 = stream2.tile([P, H], F16, tag="t2048")
            nc.vector.tensor_scalar(q2[:], q1[:], MG, 1.0,
                                    OP.subtract, OP.min)
            nc.vector.tensor_scalar(wd_sb[t][:], q2[:], -1.0, None, OP.max)

        # ============ Stage 2: attention ===================================
        # Partner-half chunk (qq=1) first per head, shipped right after its
        # deferred finish; own half (qq=0) kept in SBUF for the dense.
        mxacc = sc_tile("mxacc", (P, 1))
        cn_es = ExitStack()
        cn_pool = cn_es.enter_context(tc.tile_pool(name="cn", bufs=8))
        cn_send_pool = cn_es.enter_context(tc.tile_pool(name="cns", bufs=3))
        cn_keep = {}
        state = {"first_mx": True, "pend": None}

        def finish_half(p, rb_pool, rdr_pool):
            hh, qq, cnr, drow = p
            rdr = rdr_pool.tile([1, 1024], F32, tag="rdr")
            nc.vector.reciprocal_approx_fast(rdr[:], drow[:])
            rb = rb_pool.tile([P, 1024], F32, tag="rb")
            nc.gpsimd.partition_broadcast(rb[:], rdr[:])
            mine = (qq == 0)
            pool = cn_pool if mine else cn_send_pool
            cnf = pool.tile([P, 1024], F16, tag="cnh" if mine else "cnsd")
            nc.vector.tensor_tensor(cnf[:], cnr[:], rb[:], OP.mult)
            r = red.tile([P, 1], F32, tag="cmax")
            nc.vector.tensor_reduce(r[:], cnf[:], axis=mybir.AxisListType.X,
                                    op=OP.max, apply_absolute_value=True)
            if state["first_mx"]:
                nc.vector.tensor_copy(mxacc[:], r[:])
                state["first_mx"] = False
            else:
                nc.vector.tensor_tensor(mxacc[:], mxacc[:], r[:], OP.max)
            if mine:
                cn_keep[hh] = cnf
            else:
                # ship to partner: pair AllGather chunk, hidden under attn
                nc.sync.dma_start(exch_i[hh, :, :], cnf[:])
                nc.gpsimd.collective_compute(
                    "AllGather", OP.bypass, replica_groups=PAIRS,
                    ins=[exch_i[hh, :, :].opt()],
                    outs=[exch_o[hh, :, :, :].opt()])

        with tc.tile_pool(name="qkt", bufs=2) as qk_pool, \
             tc.tile_pool(name="vh", bufs=28) as vh_pool, \
             tc.tile_pool(name="et", bufs=20) as et_pool, \
             tc.tile_pool(name="rb", bufs=2) as rb_pool, \
             tc.tile_pool(name="cnr", bufs=3) as cnr_pool, \
             tc.tile_pool(name="rd", bufs=2) as rd_pool, \
             tc.tile_pool(name="rdr", bufs=1) as rdr_pool, \
             tc.tile_pool(name="sds", bufs=1) as sds_pool, \
             tc.tile_pool(name="ps2c", bufs=1, space="PSUM") as ps2c, \
             tc.tile_pool(name="ps2d", bufs=1, space="PSUM") as ps2d, \
             tc.tile_pool(name="ps2s", bufs=2, space="PSUM") as ps2s:
            for h in range(NHC):
                qt = qk_pool.tile([P, TOK], F16, tag="qt")
                nc.sync.dma_start(qt[:], qkt_d[h, :, :])
                kt = qk_pool.tile([P, TOK], F16, tag="kt")
                nc.sync.dma_start(kt[:], qkt_d[NHC + h, :, :])
                vh = []
                for kb in range(HB):
                    vk = vh_pool.tile([P, P], BF16, tag="vh")
                    nc.sync.dma_start(vk[:], vt_d[kb, :, h * P:(h + 1) * P])
                    vh.append(vk)

                for qq in (1, 0):
                    q0 = qq * 1024
                    et = []
                    for kb in range(HB):
                        pss = ps2s.tile([P, 1024], F32, tag="pss")
                        for sl, psl in zip(_chunks(2, 512, q0),
                                           _chunks(2, 512)):
                            nc.tensor.matmul(pss[:, psl],
                                             lhsT=kt[:, kb * P:(kb + 1) * P],
                                             rhs=qt[:, sl],
                                             start=True, stop=True)
                        e = et_pool.tile([P, 1024], BF16, tag="et")
                        nc.scalar.activation(
                            e[:], pss[:], AF.Exp,
                            bias=(mask_sb[:, kb:kb + 1] if use_mask else 0.0),
                            scale=(a2_b[:] if qk_bias_zero else INV_SQD))
                        et.append(e)

                    if state["pend"] is not None:
                        finish_half(state["pend"], rb_pool, rdr_pool)
                        state["pend"] = None

                    psc = ps2c.tile([P, 1024], F32, tag="psc")
                    psd = ps2d.tile([P, 1024], F32, tag="psd")
                    # DVE pre-clear: leaves has_written UNSET, so each
                    # col-group chain's first (start=False) matmul
                    # overwrites its rows; later ones accumulate.
                    nc.vector.memset(psd[:], 0.0)
                    for bt in range(4):
                        for kb in range(4 * bt, 4 * bt + 4):
                            vv = vh[kb][:]
                            for sl in _chunks(2, 512):
                                nc.tensor.matmul(psc[:, sl], lhsT=vv,
                                                 rhs=et[kb][:, sl],
                                                 start=(kb == 0),
                                                 stop=(kb == HB - 1))
                        # denominator: 4 ones-matmuls packed into distinct
                        # PE col-groups run concurrently
                        for sl in _chunks(2, 512):
                            for j in range(4):
                                kb = 4 * bt + j
                                nc.tensor.matmul(
                                    psd[32 * j:32 * j + 1, sl],
                                    lhsT=ones_col[:],
                                    rhs=et[kb][:, sl],
                                    start=False,
                                    stop=(bt == 3),
                                    tile_position=(0, 32 * j),
                                    skip_group_check=True)

                    cnr = cnr_pool.tile([P, 1024], F32, tag="cnr")
                    if v_bias_zero:
                        nc.vector.tensor_scalar(cnr[:], psc[:], alpv_b[:],
                                                None, OP.mult)
                    else:
                        nc.vector.tensor_copy(cnr[:], psc[:])
                    s01 = sds_pool.tile([1, 1024], F32, tag="s01")
                    nc.vector.tensor_copy(s01[:], psd[0:1, :])
                    nc.vector.tensor_tensor(s01[:], s01[:],
                                            psd[32:33, :], OP.add)
                    nc.vector.tensor_tensor(s01[:], s01[:],
                                            psd[64:65, :], OP.add)
                    drow = rd_pool.tile([1, 1024], F32, tag="rd")
                    nc.vector.tensor_tensor(drow[:], s01[:],
                                            psd[96:97, :], OP.add)
                    state["pend"] = (h, qq, cnr, drow)
                    quantize_wd_tile()
            finish_half(state["pend"], rb_pool, rdr_pool)
            state["pend"] = None


        # ============ ctx max AllReduce + quantize scales ==================
        nc.sync.dma_start(c_mc_i[:, :], mxacc[:])
        nc.gpsimd.collective_compute(
            "AllReduce", OP.max, replica_groups=ALL8,
            ins=[c_mc_i[:, :].opt()], outs=[c_mc_o[:, :].opt()])
        cm = sc_tile("cm", (P, 1))
        nc.sync.dma_start(cm[:], c_mc_o[:, :])
        cmr = sc_tile("cmr", (P, 1))
        nc.gpsimd.partition_all_reduce(cmr[:], cm[:], channels=P,
                                       reduce_op=bass_isa.ReduceOp.max)
        cm1 = sc_tile("cm1", (P, 1))
        nc.vector.tensor_scalar(cm1[:], cmr[:], 1e-8, None, OP.add)
        rcm = sc_tile("rcm", (P, 1))
        nc.vector.reciprocal(rcm[:], cm1[:])
        sctx_b = sc_tile("sctx_b", (P, 1))
        nc.vector.tensor_scalar(sctx_b[:], rcm[:], 127.0, None, OP.mult)
        ad_t = sc_tile("ad_t", (P, 1))
        nc.vector.tensor_tensor(ad_t[:], gd[:], cm1[:], OP.mult)
        alphad_b = sc_tile("alphad_b", (P, 1))
        nc.vector.tensor_scalar(alphad_b[:], ad_t[:], 1.0 / 127.0, None,
                                OP.mult)

        bdb = None
        if not d_bias_zero:
            bd_sb = smalls.tile([1, H], F32, tag="bd_sb")
            nc.sync.dma_start(bd_sb[:], bdh[:, :])
            bdb = smalls.tile([P, H], F32, tag="bdb")
            nc.gpsimd.partition_broadcast(bdb[:], bd_sb[:])

        lnwb = lnbb = None
        if not ln_trivial:
            lnw_sb = smalls.tile([1, H], F32, tag="lnw_sb")
            nc.sync.dma_start(lnw_sb[:], lnw[:, :])
            lnwb = smalls.tile([P, H], F32, tag="lnwb")
            nc.gpsimd.partition_broadcast(lnwb[:], lnw_sb[:])
            lnb_sb = smalls.tile([1, H], F32, tag="lnb_sb")
            nc.sync.dma_start(lnb_sb[:], lnb[:, :])
            lnbb = smalls.tile([P, H], F32, tag="lnbb")
            nc.gpsimd.partition_broadcast(lnbb[:], lnb_sb[:])

        # ============ Stage 3: quantize ctx (16 kb), local dense + LN ======
        with tc.tile_pool(name="cq", bufs=HB) as cq_pool, \
             tc.tile_pool(name="cnp", bufs=2) as cnp_pool, \
             tc.tile_pool(name="cnpc", bufs=NHC) as cnpc_pool, \
             tc.tile_pool(name="s3ev", bufs=2) as ev3_pool, \
             tc.tile_pool(name="s3sq", bufs=1) as sq_pool, \
             tc.tile_pool(name="lnx", bufs=1) as lnx_pool, \
             tc.tile_pool(name="lns", bufs=6) as lns_pool, \
             tc.tile_pool(name="ps3", bufs=2, space="PSUM") as ps3:
            # partner heads: fetch both AllGather slots, mask-combine on DVE
            # (no sctx dependency -> overlaps the ctx-max collective wait)
            cnp = []
            for hh in range(NHC):
                cp0 = cnp_pool.tile([P, HALF], F16, tag="cnp")
                nc.sync.dma_start(cp0[:], exch_o[hh, 0, :, :])
                cp1 = cnp_pool.tile([P, HALF], F16, tag="cnp")
                nc.sync.dma_start(cp1[:], exch_o[hh, 1, :, :])
                pc = cnpc_pool.tile([P, HALF], F16, tag="cnpc")
                t0 = stream2.tile([P, HALF], F16, tag="cq16")
                nc.vector.tensor_scalar(t0[:], cp0[:], m0_b[:], None,
                                        OP.mult)
                nc.vector.scalar_tensor_tensor(pc[:], cp1[:], m1_b[:],
                                               t0[:], OP.mult, OP.add)
                cnp.append(pc)
            ctxq = []
            for kb in range(HB):
                q = cq_pool.tile([P, HALF], F16, tag="cq")
                src = cn_keep[kb][:] if kb < NHC else cnp[kb - NHC][:]
                t1 = stream2.tile([P, HALF], F16, tag="cq16")
                if kb % 2 == 0:
                    nc.scalar.activation(t1[:], src, AF.Identity,
                                         bias=mgb[:], scale=sctx_b[:])
                else:
                    nc.vector.tensor_scalar(t1[:], src, sctx_b[:], MG,
                                            OP.mult, OP.add)
                nc.vector.tensor_scalar(q[:], t1[:], MG, None,
                                        OP.subtract)
                ctxq.append(q)

            def dense_pass(tb, psum, kbs):
                for kb in kbs:
                    for sl in _chunks(4, 512):
                        nc.tensor.matmul(
                            psum[:, sl],
                            lhsT=ctxq[kb][:, tb * P:(tb + 1) * P],
                            rhs=wd_sb[kb][:, sl],
                            start=(kb == 0), stop=(kb == HB - 1))

            # local-head (kb<8) passes need only the locally-quantized ctx,
            # so they start while the partner halves are still being
            # combined/quantized; partner passes trail by two blocks.
            psums = {}
            for t in (0, 1):
                psums[t] = ps3.tile([P, H], F32, tag="ps", name="dps%d" % t)
                dense_pass(t, psums[t], range(NHC))
            for tb in range(NHC):
                psum = psums[tb]
                dense_pass(tb, psum, range(NHC, HB))
                nt = tb + 2
                if nt < NHC:
                    psums[nt] = ps3.tile([P, H], F32, tag="ps",
                                         name="dps%d" % nt)
                    dense_pass(nt, psums[nt], range(NHC))
                # residual + layernorm straight off the psum
                x_t = lnx_pool.tile([P, H], F32, tag="lnx")
                nc.sync.dma_start(x_t[:], xr[tb * P:(tb + 1) * P, :])
                y = ev3_pool.tile([P, H], F32, tag="lny")
                ysum = lns_pool.tile([P, 1], F32, tag="ysum")
                if d_bias_zero:
                    nc.vector.scalar_tensor_tensor(
                        y[:], psum[:], alphad_b[:], x_t[:],
                        OP.mult, OP.add, accum_out=ysum[:])
                else:
                    t = sq_pool.tile([P, H], F32, tag="lnt")
                    nc.vector.scalar_tensor_tensor(
                        t[:], psum[:], alphad_b[:], bdb[:], OP.mult, OP.add)
                    nc.vector.scalar_tensor_tensor(
                        y[:], x_t[:], 1.0, t[:], OP.mult, OP.add,
                        accum_out=ysum[:])
                nmu = lns_pool.tile([P, 1], F32, tag="nmu")
                nc.vector.tensor_scalar(nmu[:], ysum[:], -1.0 / H, None,
                                        OP.mult)
                sq = sq_pool.tile([P, H], F32, tag="lnsq")
                sqs = lns_pool.tile([P, 1], F32, tag="sqs")
                nc.scalar.activation(sq[:], y[:], AF.Square,
                                     bias=nmu[:], scale=1.0,
                                     accum_out=sqs[:])
                v1 = lns_pool.tile([P, 1], F32, tag="v1")
                nc.vector.tensor_scalar(v1[:], sqs[:], 1.0 / H, LN_EPS,
                                        OP.mult, OP.add)
                v2 = lns_pool.tile([P, 1], F32, tag="v2")
                nc.vector.reciprocal(v2[:], v1[:])
                rstd = lns_pool.tile([P, 1], F32, tag="rstd")
                nc.scalar.activation(rstd[:], v2[:], AF.Sqrt)
                nmr = lns_pool.tile([P, 1], F32, tag="nmr")
                nc.vector.tensor_tensor(nmr[:], nmu[:], rstd[:], OP.mult)

                yn = ev3_pool.tile([P, H], F32, tag="lnyn")
                nc.scalar.activation(yn[:], y[:], AF.Identity,
                                     bias=nmr[:], scale=rstd[:])
                if not ln_trivial:
                    nc.vector.tensor_tensor(yn[:], yn[:], lnwb[:], OP.mult)
                    nc.vector.tensor_tensor(yn[:], yn[:], lnbb[:], OP.add)
                nc.sync.dma_start(out[tb * P:(tb + 1) * P, :], yn[:])
        cn_es.close()
        wd_es.close()


# ======================= host side =======================================

def make_in_maps(hidden_states, attention_mask, W_qkv, b_qkv, W_dense,
                 b_dense, ln_w, ln_b):
    x = np.asarray(hidden_states, dtype=np.float32)
    mask = np.asarray(attention_mask, dtype=np.float32)
    Wq = np.asarray(W_qkv, dtype=np.float32)
    bq = np.asarray(b_qkv, dtype=np.float32)
    Wd = np.asarray(W_dense, dtype=np.float32)
    bd = np.asarray(b_dense, dtype=np.float32)
    lw = np.asarray(ln_w, dtype=np.float32)
    lb = np.asarray(ln_b, dtype=np.float32)

    WdT = Wd.T.astype(np.float16)  # [h, out]
    in_maps = []
    for c in range(NCORES):
        b, g = c // 2, c % 2
        # per-core token permutation: own half first
        tord = np.r_[g * 1024:(g + 1) * 1024,
                     (1 - g) * 1024:(1 - g) * 1024 + 1024]
        sl = slice(g * 1024, (g + 1) * 1024)
        wq_g = Wq[sl, :]
        wk_g = Wq[2048 + g * 1024:2048 + (g + 1) * 1024, :]
        wv_g = Wq[4096 + g * 1024:4096 + (g + 1) * 1024, :]
        bq_g = bq[sl]
        bk_g = bq[2048 + g * 1024:2048 + (g + 1) * 1024]
        bv_g = bq[4096 + g * 1024:4096 + (g + 1) * 1024]
        W2 = np.concatenate([wq_g, wk_g], axis=0).T  # [h, out]
        wq_tiled = np.ascontiguousarray(
            W2.reshape(16, P, 16, P).transpose(2, 1, 0, 3)
            .reshape(16, P, H).astype(np.float16))
        # W_dense^T rows: own head half first, then partner's
        wdt_c = np.ascontiguousarray(np.concatenate(
            [WdT[g * 1024:(g + 1) * 1024, :],
             WdT[(1 - g) * 1024:(1 - g) * 1024 + 1024, :]], axis=0))
        in_maps.append({
            "xt": np.ascontiguousarray(x[b].T[:, tord].astype(np.float16)),
            "xr": np.ascontiguousarray(x[b, g * 1024:(g + 1) * 1024, :]),
            "wqkt": wq_tiled,
            "wvt": np.ascontiguousarray(wv_g.T.astype(np.float16)),
            "bqk": np.ascontiguousarray(
                np.concatenate([bq_g, bk_g]).reshape(16, P).T),
            "bv": bv_g.reshape(1, 1024).copy(),
            "wdt": wdt_c,
            "bdh": bd.reshape(1, H).copy(),
            "maskt": np.ascontiguousarray(
                mask[b, 0, 0, tord].reshape(HB, P).T),
            "csel": np.eye(16, dtype=np.float32)[2 + c].reshape(1, 16).copy(),
            "pmsk": np.array([[float(g), float(1 - g)]], dtype=np.float32),
            "lnw": lw.reshape(1, H).copy(),
            "lnb": lb.reshape(1, H).copy(),
        })
    return in_maps


def build_flags(attention_mask, b_qkv, b_dense, ln_w, ln_b):
    return (
        bool(np.any(np.asarray(attention_mask) != 0.0)),
        bool(np.all(np.asarray(b_qkv)[:4096] == 0.0)),
        bool(np.all(np.asarray(b_qkv)[4096:] == 0.0)),
        bool(np.all(np.asarray(b_dense) == 0.0)),
        bool(np.all(np.asarray(ln_w) == 1.0) and np.all(np.asarray(ln_b) == 0.0)),
    )


def assemble_output(results):
    full = np.empty((B, S, H), dtype=np.float32)
    for c in range(NCORES):
        b, g = c // 2, c % 2
        full[b, g * 1024:(g + 1) * 1024, :] = results[c]["out"]
    return full


_CACHE = {}


def _get_program(flags):
    if flags not in _CACHE:
        _CACHE[flags] = build_program(*flags)
    return _CACHE[flags]


def _ensure_ntff_hook():
    """Provide antenv.axon_hooks (missing in this image) so trace=True can
    capture NTFF profiles through the axon PJRT plugin."""
    import types

    try:
        import antenv.axon_hooks  # noqa: F401
        return
    except ImportError:
        pass
    try:
        import antenv
    except ImportError:
        return
    mod = types.ModuleType("antenv.axon_hooks")
    holder = {"h": None}
    mod.set_axon_ntff_profile_hook = lambda h: holder.__setitem__("h", h)
    mod.get_axon_ntff_profile_hook = lambda: holder["h"]
    sys.modules["antenv.axon_hooks"] = mod
    antenv.axon_hooks = mod
    try:
        if "/root/.axon_site" not in sys.path:
            sys.path.insert(0, "/root/.axon_site")
        from trn_agent_boot.trn_boot import _ntff_profile_via_ctypes
        h = _ntff_profile_via_ctypes("/opt/axon/libaxon_pjrt.so")
        if h is not None:
            mod.set_axon_ntff_profile_hook(h)
    except Exception:
        pass


def kernel(hidden_states, attention_mask, W_qkv, b_qkv, W_dense, b_dense,
           ln_w, ln_b, trace=False):
    from concourse.bass_utils import run_bass_kernel_spmd

    flags = build_flags(attention_mask, b_qkv, b_dense, ln_w, ln_b)
    nc = _get_program(flags)
    in_maps = make_in_maps(hidden_states, attention_mask, W_qkv, b_qkv,
                           W_dense, b_dense, ln_w, ln_b)
    if trace:
        _ensure_ntff_hook()
        try:
            res = run_bass_kernel_spmd(nc, in_maps,
                                       core_ids=list(range(NCORES)),
                                       trace=True)
        except Exception as e:
            print("trace run failed (%s); retrying untraced" % e)
            res = run_bass_kernel_spmd(nc, in_maps,
                                       core_ids=list(range(NCORES)),
                                       trace=False)
    else:
        res = run_bass_kernel_spmd(nc, in_maps, core_ids=list(range(NCORES)),
                                   trace=False)
    out = assemble_output(res.results)
    kernel.last_result = res
    return out


# revision 41
# speedup vs baseline: 1.1434x; 1.1434x over previous
"""BitNet attention layer (quantized QKV + attention + quantized dense + LN)
as a Bass/Tile SPMD kernel for 8 Trainium2 NeuronCores.

Sharding: core c = 2*b + g handles batch b (of 4) and head-group g (of 2,
8 heads each).  The host permutes the token axis per core so each core's
own 1024 tokens sit in columns 0:1024 (making the program g-independent):
QKV projection + attention are fully local; after each partner-half
attention chunk finishes, it is shipped to the paired core via a pair
AllGather hidden under the remaining attention matmuls.  The dense output
projection then runs with the FULL 2048-deep contraction on each core's
own token half (host also permutes W_dense rows to [own heads, partner
heads]) -- no output ReduceScatter and no serial tail: residual+layernorm
stream right behind the dense matmuls.  The partner AllGather slot is
selected with a host-fed {0,1} mask pair folded into the ctx quantize.

The softmax denominator (a partition-axis reduction) is computed with
ones-column matmuls packed 4-to-a-pass into distinct PE col-groups via
tile_position, costing ~1/4 of a full matmul stream; the 4 partial rows
are combined by DVE reads of the PSUM rows.  Row reciprocals use the fast
custom-DVE approximation (~18 bits).

Numerics: activations are round()ed to ints in [-127,127] and weights to
{-1,0,1} ({-2,0,2} for the sign-quantized W_v/W_d, 0.5 folded into the
dequant scales); all exactly representable in f16, and f32 PSUM
accumulation of <=2048 such products is exact.  With zero biases (the
benchmark instance) projection outputs stay RAW integer sums and the
dequant scales fold downstream.  Magic-round constant 1536 keeps q+magic
inside the f16 ulp=1 binade for both signs, so rounding matches
jnp.round exactly.
"""

import math
import sys

import numpy as np

sys.path.insert(0, "/opt/trn_rl_repo")

import concourse.bacc as bacc
import concourse.bass as bass
import concourse.bass_isa as bass_isa
import concourse.mybir as mybir
import concourse.tile as tile

F32 = mybir.dt.float32
F16 = mybir.dt.float16
BF16 = mybir.dt.bfloat16
AF = mybir.ActivationFunctionType
OP = mybir.AluOpType

P = 128
H = 2048
S = 2048
B = 4
NH = 16
HD = 128
NCORES = 8
TOK = S
HB = H // P            # 16 h blocks
NHC = NH // 2          # 8 heads per core
HALF = TOK // 2        # 1024 tokens kept per core
MG = 1536.0            # f16 magic: q+MG stays in [1024,2048) => ulp 1
INV_SQD = 1.0 / math.sqrt(HD)
LN_EPS = 1e-5
PAIRS = [[0, 1], [2, 3], [4, 5], [6, 7]]
ALL8 = [list(range(NCORES))]


def _chunks(count, width, base=0):
    return [slice(base + i * width, base + (i + 1) * width) for i in range(count)]


def build_program(use_mask: bool, qk_bias_zero: bool, v_bias_zero: bool,
                  d_bias_zero: bool, ln_trivial: bool):
    nc = bacc.Bacc("TRN2", target_bir_lowering=False, debug=False,
                   enable_asserts=False, num_devices=NCORES)

    # ---- I/O (token axis per-core permuted: own half first) --------------
    xt = nc.dram_tensor("xt", [H, TOK], F16, kind="ExternalInput")
    xr = nc.dram_tensor("xr", [HALF, H], F32, kind="ExternalInput")
    # W_qk^T column-slab-tiled: [ob, p, kb*128+c] = W^T[kb*128+p, ob*128+c]
    wqkt = nc.dram_tensor("wqkt", [16, P, 2048], F16, kind="ExternalInput")
    wvt = nc.dram_tensor("wvt", [H, 1024], F16, kind="ExternalInput")
    bqk = nc.dram_tensor("bqk", [P, 16], F32, kind="ExternalInput")
    bv = nc.dram_tensor("bv", [1, 1024], F32, kind="ExternalInput")
    # FULL W_dense^T, rows permuted to [own head half, partner head half]
    wdt = nc.dram_tensor("wdt", [H, H], F16, kind="ExternalInput")
    bdh = nc.dram_tensor("bdh", [1, H], F32, kind="ExternalInput")
    maskt = nc.dram_tensor("maskt", [P, HB], F32, kind="ExternalInput")
    csel = nc.dram_tensor("csel", [1, 16], F32, kind="ExternalInput")
    pmsk = nc.dram_tensor("pmsk", [1, 2], F32, kind="ExternalInput")
    lnw = nc.dram_tensor("lnw", [1, H], F32, kind="ExternalInput")
    lnb = nc.dram_tensor("lnb", [1, H], F32, kind="ExternalInput")
    out = nc.dram_tensor("out", [HALF, H], F32, kind="ExternalOutput")

    # ---- DRAM scratch ----------------------------------------------------
    qkt_d = nc.dram_tensor("qkt_d", [16, P, TOK], F16)
    vt_d = nc.dram_tensor("vt_d", [16, P, 1024], BF16)
    exch_i = nc.dram_tensor("exch_i", [NHC, P, HALF], F16)
    exch_o = nc.dram_tensor("exch_o", [NHC, 2, P, HALF], F16)
    c_add_i = nc.dram_tensor("c_add_i", [P, 1], F32)
    c_add_o = nc.dram_tensor("c_add_o", [P, 1], F32)
    c_mx_i = nc.dram_tensor("c_mx_i", [P, 1], F32)
    c_mx_o = nc.dram_tensor("c_mx_o", [P, 1], F32)
    c_mc_i = nc.dram_tensor("c_mc_i", [P, 1], F32)
    c_mc_o = nc.dram_tensor("c_mc_o", [P, 1], F32)
    c_wu_i = nc.dram_tensor("c_wu_i", [1, 16], F32)
    c_wu_o = nc.dram_tensor("c_wu_o", [1, 16], F32)

    with tile.TileContext(nc) as tc:
        _emit(tc, locals(), use_mask, qk_bias_zero, v_bias_zero,
              d_bias_zero, ln_trivial)

    nc.compile()
    return nc


def _emit(tc, T, use_mask, qk_bias_zero, v_bias_zero, d_bias_zero,
          ln_trivial):
    nc = tc.nc
    xt, xr, wqkt, wvt, bqk, bv, wdt, bdh = (T["xt"], T["xr"], T["wqkt"],
                                            T["wvt"], T["bqk"], T["bv"],
                                            T["wdt"], T["bdh"])
    maskt, lnw, lnb, out = T["maskt"], T["lnw"], T["lnb"], T["out"]
    csel, pmsk = T["csel"], T["pmsk"]
    qkt_d, vt_d = T["qkt_d"], T["vt_d"]
    exch_i, exch_o = T["exch_i"], T["exch_o"]
    c_add_i, c_add_o = T["c_add_i"], T["c_add_o"]
    c_mx_i, c_mx_o = T["c_mx_i"], T["c_mx_o"]
    c_mc_i, c_mc_o = T["c_mc_i"], T["c_mc_o"]
    c_wu_i, c_wu_o = T["c_wu_i"], T["c_wu_o"]

    from contextlib import ExitStack

    est = ExitStack()
    with est:
        smalls = est.enter_context(tc.tile_pool(name="smalls", bufs=1))
        stream2 = est.enter_context(tc.tile_pool(name="stream2", bufs=2))
        red = est.enter_context(tc.tile_pool(name="red", bufs=4))

        def sc_tile(name, shape=(1, 1)):
            return smalls.tile(list(shape), F32, tag=name, name=name)

        def bcast(name, src):
            b = sc_tile(name, (P, 1))
            nc.gpsimd.partition_broadcast(b[:], src[:])
            return b

        ones_col = smalls.tile([P, 1], BF16, tag="ones_col")
        nc.vector.memset(ones_col[:], 1.0)
        mgb = smalls.tile([P, 1], F32, tag="mgb")
        nc.vector.memset(mgb[:], MG)
        csb = smalls.tile([1, 16], F32, tag="csb")
        nc.sync.dma_start(csb[:], csel[:, :])
        pms = smalls.tile([1, 2], F32, tag="pms")
        nc.sync.dma_start(pms[:], pmsk[:, :])
        m0 = sc_tile("m0")
        nc.vector.tensor_copy(m0[:], pms[0:1, 0:1])
        m1 = sc_tile("m1")
        nc.vector.tensor_copy(m1[:], pms[0:1, 1:2])
        m0_b = bcast("m0_b", m0)
        m1_b = bcast("m1_b", m1)
        bqk_sb = None
        if not qk_bias_zero:
            bqk_sb = smalls.tile([P, 16], F32, tag="bqk_sb")
            nc.sync.dma_start(bqk_sb[:], bqk[:, :])
        mask_sb = None
        if use_mask:
            mask_sb = smalls.tile([P, HB], F32, tag="mask_sb")
            nc.sync.dma_start(mask_sb[:], maskt[:, :])

        # ============ Stage 0a: load x, max|x| -> AR_x =====================
        s1es = ExitStack()
        xq_pool = s1es.enter_context(tc.tile_pool(name="xq", bufs=HB))
        wda_es = ExitStack()
        wda_pool = wda_es.enter_context(tc.tile_pool(name="wda", bufs=2))
        xq = []
        xmax = sc_tile("xmax", (P, 1))
        for t in range(HB):
            xf = xq_pool.tile([P, TOK], F16, tag="xq")
            nc.sync.dma_start(xf[:], xt[t * P:(t + 1) * P, :])
            xq.append(xf)
            r = red.tile([P, 1], F32, tag="xred")
            nc.vector.tensor_reduce(r[:], xf[:], axis=mybir.AxisListType.X,
                                    op=OP.max, apply_absolute_value=True)
            if t == 0:
                nc.vector.tensor_copy(xmax[:], r[:])
            else:
                nc.vector.tensor_tensor(xmax[:], xmax[:], r[:], OP.max)
        nc.sync.dma_start(c_mx_i[:, :], xmax[:])
        nc.gpsimd.collective_compute(
            "AllReduce", OP.max, replica_groups=ALL8,
            ins=[c_mx_i[:, :].opt()], outs=[c_mx_o[:, :].opt()])

        # ============ Stage 0b: load W_qk + W_v, |W| abs -> AR_A ===========
        accA = sc_tile("accA", (P, 1))
        ps0 = ExitStack()
        ps0_pool = ps0.enter_context(tc.tile_pool(name="ps0", bufs=1,
                                                  space="PSUM"))
        absdump = ps0_pool.tile([P, 2048], F32, tag="absdump")

        wv_es = ExitStack()
        wv_pool = wv_es.enter_context(tc.tile_pool(name="wv16", bufs=HB))
        wq_es = ExitStack()
        wq_pool = wq_es.enter_context(tc.tile_pool(name="wq16", bufs=HB))

        wq16 = []
        for ob in range(HB):
            w16 = wq_pool.tile([P, 2048], F16, tag="wq16")
            nc.sync.dma_start(w16[:], wqkt[ob, :, :])
            r = red.tile([P, 1], F32, tag="wred")
            if ob < 8:
                nc.scalar.activation(absdump[:], w16[:], AF.Abs,
                                     accum_out=r[:])
            else:
                nc.vector.tensor_reduce(r[:], w16[:],
                                        axis=mybir.AxisListType.X,
                                        op=OP.add, apply_absolute_value=True)
            if ob == 0:
                nc.vector.tensor_copy(accA[:], r[:])
            else:
                nc.vector.tensor_tensor(accA[:], accA[:], r[:], OP.add)
            wq16.append(w16)

        wv16 = []
        for t in range(HB):
            wf = wv_pool.tile([P, 1024], F16, tag="wv16")
            nc.sync.dma_start(wf[:], wvt[t * P:(t + 1) * P, :])
            r = red.tile([P, 1], F32, tag="wred")
            nc.vector.tensor_reduce(r[:], wf[:], axis=mybir.AxisListType.X,
                                    op=OP.add, apply_absolute_value=True)
            nc.vector.tensor_tensor(accA[:], accA[:], r[:], OP.add)
            wv16.append(wf)

        nc.sync.dma_start(c_add_i[:, :], accA[:])
        nc.gpsimd.collective_compute(
            "AllReduce", OP.add, replica_groups=ALL8,
            ins=[c_add_i[:, :].opt()], outs=[c_add_o[:, :].opt()])

        ps0.close()

        # ============ scales (x first: xq is on the critical path) =========
        xm = sc_tile("xm", (P, 1))
        nc.sync.dma_start(xm[:], c_mx_o[:, :])
        xmaxr = sc_tile("xmaxr", (P, 1))
        nc.gpsimd.partition_all_reduce(xmaxr[:], xm[:], channels=P,
                                       reduce_op=bass_isa.ReduceOp.max)
        xm1 = sc_tile("xm1", (P, 1))
        nc.vector.tensor_scalar(xm1[:], xmaxr[:], 1e-8, None, OP.add)
        rxm = sc_tile("rxm", (P, 1))
        nc.vector.reciprocal(rxm[:], xm1[:])
        sx_b = sc_tile("sx_b", (P, 1))
        nc.vector.tensor_scalar(sx_b[:], rxm[:], 127.0, None, OP.mult)

        # quantize x IN PLACE: round(x*sx) -> f16 ints (Act/DVE split)
        for kb in range(HB):
            t1 = stream2.tile([P, TOK], F16, tag="t2048")
            if kb < 8:
                nc.scalar.activation(t1[:], xq[kb][:], AF.Identity,
                                     bias=mgb[:], scale=sx_b[:])
            else:
                nc.vector.tensor_scalar(t1[:], xq[kb][:], sx_b[:], MG,
                                        OP.mult, OP.add)
            nc.vector.tensor_scalar(xq[kb][:], t1[:], MG, None, OP.subtract)

        # gamma_qkv = sum|W_qkv|/(3H*H)+1e-5 (all-8 add = 4x full sum)
        wsA = sc_tile("wsA", (P, 1))
        nc.sync.dma_start(wsA[:], c_add_o[:, :])
        accAr = sc_tile("accAr", (P, 1))
        nc.gpsimd.partition_all_reduce(accAr[:], wsA[:], channels=P,
                                       reduce_op=bass_isa.ReduceOp.add)
        gq = sc_tile("gq", (P, 1))
        nc.vector.tensor_scalar(gq[:], accAr[:],
                                1.0 / (4 * 3 * H * H), 1e-5, OP.mult, OP.add)
        igq_b = sc_tile("igq_b", (P, 1))
        nc.vector.reciprocal(igq_b[:], gq[:])

        # remaining stage-1/2 scales
        al_t = sc_tile("al_t", (P, 1))
        nc.vector.tensor_tensor(al_t[:], gq[:], xm1[:], OP.mult)
        alpha_b = sc_tile("alpha_b", (P, 1))
        nc.vector.tensor_scalar(alpha_b[:], al_t[:], 1.0 / 127.0, None,
                                OP.mult)
        a2_b = sc_tile("a2_b", (P, 1))
        nc.vector.tensor_tensor(a2_b[:], alpha_b[:], alpha_b[:], OP.mult)
        nc.vector.tensor_scalar(a2_b[:], a2_b[:], INV_SQD, None, OP.mult)
        # sign-route (W_v) gives {-2,0,2}; alpv carries the 0.5
        alpv_b = sc_tile("alpv_b", (P, 1))
        nc.vector.tensor_scalar(alpv_b[:], alpha_b[:], 0.5, None, OP.mult)
        ntq_b = sc_tile("ntq_b", (P, 1))
        nc.vector.tensor_scalar(ntq_b[:], gq[:], -0.5, None, OP.mult)
        ptq_b = sc_tile("ptq_b", (P, 1))
        nc.vector.tensor_scalar(ptq_b[:], gq[:], 0.5, None, OP.mult)

        bvb = None
        if not v_bias_zero:
            bv_sb = smalls.tile([1, 1024], F32, tag="bv_sb")
            nc.sync.dma_start(bv_sb[:], bv[:, :])
            bvb = smalls.tile([P, 1024], F32, tag="bvb")
            nc.gpsimd.partition_broadcast(bvb[:], bv_sb[:])

        # ============ W_d abs pass (streamed; overlaps QK) =================
        # wda pool stays open through stage 1 so its addresses are not
        # recycled by the QK eviction pool (address reuse would serialize
        # the evictions behind this DMA-paced pass).
        accD = sc_tile("accD", (P, 1))

        def wv_sign_unit(t, sgv_pool):
            s1 = sgv_pool.tile([P, 1024], F16, tag="sg16")
            nc.scalar.activation(s1[:], wv16[t][:], AF.Sign, bias=ntq_b[:])
            s2 = sgv_pool.tile([P, 1024], F16, tag="sg16")
            nc.scalar.activation(s2[:], wv16[t][:], AF.Sign, bias=ptq_b[:])
            nc.vector.tensor_tensor(wv16[t][:], s1[:], s2[:], OP.add)

        def wda_abs_unit(t):
            wt = wda_pool.tile([P, H], F16, tag="wda", name="wda%d" % t)
            nc.sync.dma_start(wt[:], wdt[t * P:(t + 1) * P, :])
            r = red.tile([P, 1], F32, tag="dred")
            nc.scalar.activation(wt[:], wt[:], AF.Abs, accum_out=r[:])
            if t == 0:
                nc.gpsimd.tensor_copy(accD[:], r[:])
            else:
                nc.gpsimd.tensor_tensor(accD[:], accD[:], r[:], OP.add)
        # ============ Stage 1: QK projection (lazy W quantize) =============
        with tc.tile_pool(name="s1ev", bufs=2) as ev_pool, \
             tc.tile_pool(name="sgv", bufs=3) as sgv_pool, \
             tc.tile_pool(name="ps1", bufs=2, space="PSUM") as ps1:
            for ob in range(16):
                wv_sign_unit(ob, sgv_pool)
                wda_abs_unit(ob)
                # ternary round(w/gq), clip [-1,1], in place (DVE magic)
                t1 = stream2.tile([P, 2048], F16, tag="t2048")
                nc.vector.tensor_scalar(t1[:], wq16[ob][:], igq_b[:], MG,
                                        OP.mult, OP.add)
                t2 = stream2.tile([P, 2048], F16, tag="t2048")
                nc.vector.tensor_scalar(t2[:], t1[:], MG, 1.0,
                                        OP.subtract, OP.min)
                nc.vector.tensor_scalar(wq16[ob][:], t2[:], -1.0, None,
                                        OP.max)
                psum = ps1.tile([P, TOK], F32, tag="ps")
                for kb in range(HB):
                    for sl in _chunks(4, 512):
                        nc.tensor.matmul(psum[:, sl],
                                         lhsT=wq16[ob][:, kb * P:(kb + 1) * P],
                                         rhs=xq[kb][:, sl],
                                         start=(kb == 0), stop=(kb == HB - 1))
                ev = ev_pool.tile([P, TOK], F16, tag="ev")
                if qk_bias_zero:
                    nc.scalar.activation(ev[:], psum[:], AF.Identity,
                                         bias=0.0, scale=1.0)
                else:
                    nc.scalar.activation(ev[:], psum[:], AF.Identity,
                                         bias=bqk_sb[:, ob:ob + 1],
                                         scale=alpha_b[:])
                nc.sync.dma_start(qkt_d[ob, :, :], ev[:])
        wq_es.close()

        with tc.tile_pool(name="evv", bufs=3) as evv_pool, \
             tc.tile_pool(name="ps1v", bufs=2, space="PSUM") as ps1v:
            for tb in range(HB):
                psum = ps1v.tile([P, 1024], F32, tag="ps")
                for kb in range(HB):
                    for sl in _chunks(2, 512):
                        nc.tensor.matmul(
                            psum[:, sl],
                            lhsT=xq[kb][:, tb * P:(tb + 1) * P],
                            rhs=wv16[kb][:, sl],
                            start=(kb == 0), stop=(kb == HB - 1))
                v = evv_pool.tile([P, 1024], BF16, tag="vt")
                if v_bias_zero:
                    nc.scalar.activation(v[:], psum[:], AF.Identity,
                                         bias=0.0, scale=1.0)
                else:
                    nc.vector.scalar_tensor_tensor(v[:], psum[:], alpv_b[:],
                                                   bvb[:], OP.mult, OP.add)
                nc.sync.dma_start(vt_d[tb, :, :], v[:])
        wv_es.close()
        wda_es.close()
        s1es.close()

        accDr = sc_tile("accDr", (P, 1))
        nc.gpsimd.partition_all_reduce(accDr[:], accD[:], channels=P,
                                       reduce_op=bass_isa.ReduceOp.add)
        gd = sc_tile("gd", (P, 1))
        nc.vector.tensor_scalar(gd[:], accDr[:],
                                1.0 / (H * H), 1e-5, OP.mult, OP.add)
        igd_b = sc_tile("igd_b", (P, 1))
        nc.vector.reciprocal(igd_b[:], gd[:])

        # ============ W_d load (quantize deferred into attention) ==========
        wd_es = ExitStack()
        wd_pool = wd_es.enter_context(tc.tile_pool(name="wd_sb", bufs=HB))
        wd_sb = []
        for t in range(HB):
            w = wd_pool.tile([P, H], F16, tag="wd_sb", name="wd%d" % t)
            # software-DGE queue: keeps this 8MB burst off the HWDGE
            # queues that carry the latency-critical qt/kt/vh loads
            nc.gpsimd.dma_start(w[:], wdt[t * P:(t + 1) * P, :])
            wd_sb.append(w)
        wd_jobs = list(range(HB))

        def quantize_wd_tile():
            # one W_d ternary round(w/gd) per attention chunk, on DVE
            if not wd_jobs:
                return
            t = wd_jobs.pop(0)
            q1 = stream2.tile([P, H], F16, tag="t2048")
            nc.vector.tensor_scalar(q1[:], wd_sb[t][:], igd_b[:], MG,
                                    OP.mult, OP.add)
            q2 = stream2.tile([P, H], F16, tag="t2048")
            nc.vector.tensor_scalar(q2[:], q1[:], MG, 1.0,
                                    OP.subtract, OP.min)
            nc.vector.tensor_scalar(wd_sb[t][:], q2[:], -1.0, None, OP.max)

        # ============ Stage 2: attention ===================================
        # Partner-half chunk (qq=1) first per head, shipped right after its
        # deferred finish; own half (qq=0) kept in SBUF for the dense.
        mxacc = sc_tile("mxacc", (P, 1))
        cn_es = ExitStack()
        cn_pool = cn_es.enter_context(tc.tile_pool(name="cn", bufs=8))
        cn_send_pool = cn_es.enter_context(tc.tile_pool(name="cns", bufs=3))
        cn_keep = {}
        state = {"first_mx": True, "pend": None}

        def finish_half(p, rb_pool, rdr_pool):
            hh, qq, cnr, drow = p
            rdr = rdr_pool.tile([1, 1024], F32, tag="rdr")
            nc.vector.reciprocal_approx_fast(rdr[:], drow[:])
            rb = rb_pool.tile([P, 1024], F32, tag="rb")
            nc.gpsimd.partition_broadcast(rb[:], rdr[:])
            mine = (qq == 0)
            pool = cn_pool if mine else cn_send_pool
            cnf = pool.tile([P, 1024], F16, tag="cnh" if mine else "cnsd")
            nc.vector.tensor_tensor(cnf[:], cnr[:], rb[:], OP.mult)
            r = red.tile([P, 1], F32, tag="cmax")
            nc.vector.tensor_reduce(r[:], cnf[:], axis=mybir.AxisListType.X,
                                    op=OP.max, apply_absolute_value=True)
            if state["first_mx"]:
                nc.vector.tensor_copy(mxacc[:], r[:])
                state["first_mx"] = False
            else:
                nc.vector.tensor_tensor(mxacc[:], mxacc[:], r[:], OP.max)
            if mine:
                cn_keep[hh] = cnf
            else:
                # ship to partner: pair AllGather chunk, hidden under attn
                nc.sync.dma_start(exch_i[hh, :, :], cnf[:])
                nc.gpsimd.collective_compute(
                    "AllGather", OP.bypass, replica_groups=PAIRS,
                    ins=[exch_i[hh, :, :].opt()],
                    outs=[exch_o[hh, :, :, :].opt()])

        with tc.tile_pool(name="qkt", bufs=2) as qk_pool, \
             tc.tile_pool(name="vh", bufs=28) as vh_pool, \
             tc.tile_pool(name="et", bufs=18) as et_pool, \
             tc.tile_pool(name="rb", bufs=2) as rb_pool, \
             tc.tile_pool(name="cnr", bufs=2) as cnr_pool, \
             tc.tile_pool(name="rd", bufs=2) as rd_pool, \
             tc.tile_pool(name="rdr", bufs=1) as rdr_pool, \
             tc.tile_pool(name="sds", bufs=1) as sds_pool, \
             tc.tile_pool(name="ps2c", bufs=1, space="PSUM") as ps2c, \
             tc.tile_pool(name="ps2d", bufs=1, space="PSUM") as ps2d, \
             tc.tile_pool(name="ps2s", bufs=2, space="PSUM") as ps2s:
            for h in range(NHC):
                qt = qk_pool.tile([P, TOK], F16, tag="qt")
                nc.sync.dma_start(qt[:], qkt_d[h, :, :])
                kt = qk_pool.tile([P, TOK], F16, tag="kt")
                nc.sync.dma_start(kt[:], qkt_d[NHC + h, :, :])
                vh = []
                for kb in range(HB):
                    vk = vh_pool.tile([P, P], BF16, tag="vh")
                    nc.sync.dma_start(vk[:], vt_d[kb, :, h * P:(h + 1) * P])
                    vh.append(vk)

                for qq in (1, 0):
                    q0 = qq * 1024
                    et = []
                    for kb in range(HB):
                        pss = ps2s.tile([P, 1024], F32, tag="pss")
                        for sl, psl in zip(_chunks(2, 512, q0),
                                           _chunks(2, 512)):
                            nc.tensor.matmul(pss[:, psl],
                                             lhsT=kt[:, kb * P:(kb + 1) * P],
                                             rhs=qt[:, sl],
                                             start=True, stop=True)
                        e = et_pool.tile([P, 1024], BF16, tag="et")
                        nc.scalar.activation(
                            e[:], pss[:], AF.Exp,
                            bias=(mask_sb[:, kb:kb + 1] if use_mask else 0.0),
                            scale=(a2_b[:] if qk_bias_zero else INV_SQD))
                        et.append(e)

                    if state["pend"] is not None:
                        finish_half(state["pend"], rb_pool, rdr_pool)
                        state["pend"] = None

                    psc = ps2c.tile([P, 1024], F32, tag="psc")
                    psd = ps2d.tile([P, 1024], F32, tag="psd")
                    # DVE pre-clear: leaves has_written UNSET, so each
                    # col-group chain's first (start=False) matmul
                    # overwrites its rows; later ones accumulate.
                    nc.vector.memset(psd[:], 0.0)
                    for bt in range(4):
                        for kb in range(4 * bt, 4 * bt + 4):
                            vv = vh[kb][:]
                            for sl in _chunks(2, 512):
                                nc.tensor.matmul(psc[:, sl], lhsT=vv,
                                                 rhs=et[kb][:, sl],
                                                 start=(kb == 0),
                                                 stop=(kb == HB - 1))
                        # denominator: 4 ones-matmuls packed into distinct
                        # PE col-groups run concurrently
                        for sl in _chunks(2, 512):
                            for j in range(4):
                                kb = 4 * bt + j
                                nc.tensor.matmul(
                                    psd[32 * j:32 * j + 1, sl],
                                    lhsT=ones_col[:],
                                    rhs=et[kb][:, sl],
                                    start=False,
                                    stop=(bt == 3),
                                    tile_position=(0, 32 * j),
                                    skip_group_check=True)

                    cnr = cnr_pool.tile([P, 1024], F32, tag="cnr")
                    if v_bias_zero:
                        nc.vector.tensor_scalar(cnr[:], psc[:], alpv_b[:],
                                                None, OP.mult)
                    else:
                        nc.vector.tensor_copy(cnr[:], psc[:])
                    s01 = sds_pool.tile([1, 1024], F32, tag="s01")
                    nc.vector.tensor_copy(s01[:], psd[0:1, :])
                    nc.vector.tensor_tensor(s01[:], s01[:],
                                            psd[32:33, :], OP.add)
                    nc.vector.tensor_tensor(s01[:], s01[:],
                                            psd[64:65, :], OP.add)
                    drow = rd_pool.tile([1, 1024], F32, tag="rd")
                    nc.vector.tensor_tensor(drow[:], s01[:],
                                            psd[96:97, :], OP.add)
                    state["pend"] = (h, qq, cnr, drow)
                    quantize_wd_tile()
            finish_half(state["pend"], rb_pool, rdr_pool)
            state["pend"] = None


        # ============ ctx max AllReduce + quantize scales ==================
        nc.sync.dma_start(c_mc_i[:, :], mxacc[:])
        nc.gpsimd.collective_compute(
            "AllReduce", OP.max, replica_groups=ALL8,
            ins=[c_mc_i[:, :].opt()], outs=[c_mc_o[:, :].opt()])
        cm = sc_tile("cm", (P, 1))
        nc.sync.dma_start(cm[:], c_mc_o[:, :])
        cmr = sc_tile("cmr", (P, 1))
        nc.gpsimd.partition_all_reduce(cmr[:], cm[:], channels=P,
                                       reduce_op=bass_isa.ReduceOp.max)
        cm1 = sc_tile("cm1", (P, 1))
        nc.vector.tensor_scalar(cm1[:], cmr[:], 1e-8, None, OP.add)
        rcm = sc_tile("rcm", (P, 1))
        nc.vector.reciprocal(rcm[:], cm1[:])
        sctx_b = sc_tile("sctx_b", (P, 1))
        nc.vector.tensor_scalar(sctx_b[:], rcm[:], 127.0, None, OP.mult)
        ad_t = sc_tile("ad_t", (P, 1))
        nc.vector.tensor_tensor(ad_t[:], gd[:], cm1[:], OP.mult)
        alphad_b = sc_tile("alphad_b", (P, 1))
        nc.vector.tensor_scalar(alphad_b[:], ad_t[:], 1.0 / 127.0, None,
                                OP.mult)

        bdb = None
        if not d_bias_zero:
            bd_sb = smalls.tile([1, H], F32, tag="bd_sb")
            nc.sync.dma_start(bd_sb[:], bdh[:, :])
            bdb = smalls.tile([P, H], F32, tag="bdb")
            nc.gpsimd.partition_broadcast(bdb[:], bd_sb[:])

        lnwb = lnbb = None
        if not ln_trivial:
            lnw_sb = smalls.tile([1, H], F32, tag="lnw_sb")
            nc.sync.dma_start(lnw_sb[:], lnw[:, :])
            lnwb = smalls.tile([P, H], F32, tag="lnwb")
            nc.gpsimd.partition_broadcast(lnwb[:], lnw_sb[:])
            lnb_sb = smalls.tile([1, H], F32, tag="lnb_sb")
            nc.sync.dma_start(lnb_sb[:], lnb[:, :])
            lnbb = smalls.tile([P, H], F32, tag="lnbb")
            nc.gpsimd.partition_broadcast(lnbb[:], lnb_sb[:])

        # ============ Stage 3: quantize ctx (16 kb), local dense + LN ======
        with tc.tile_pool(name="cq", bufs=HB) as cq_pool, \
             tc.tile_pool(name="cnp", bufs=2) as cnp_pool, \
             tc.tile_pool(name="cnpc", bufs=NHC) as cnpc_pool, \
             tc.tile_pool(name="s3ev", bufs=2) as ev3_pool, \
             tc.tile_pool(name="s3sq", bufs=1) as sq_pool, \
             tc.tile_pool(name="lnx", bufs=1) as lnx_pool, \
             tc.tile_pool(name="lns", bufs=6) as lns_pool, \
             tc.tile_pool(name="ps3", bufs=2, space="PSUM") as ps3:
            # partner heads: fetch both AllGather slots, mask-combine on DVE
            # (no sctx dependency -> overlaps the ctx-max collective wait)
            cnp = []
            for hh in range(NHC):
                cp0 = cnp_pool.tile([P, HALF], F16, tag="cnp")
                nc.sync.dma_start(cp0[:], exch_o[hh, 0, :, :])
                cp1 = cnp_pool.tile([P, HALF], F16, tag="cnp")
                nc.sync.dma_start(cp1[:], exch_o[hh, 1, :, :])
                pc = cnpc_pool.tile([P, HALF], F16, tag="cnpc")
                t0 = stream2.tile([P, HALF], F16, tag="cq16")
                nc.vector.tensor_scalar(t0[:], cp0[:], m0_b[:], None,
                                        OP.mult)
                nc.vector.scalar_tensor_tensor(pc[:], cp1[:], m1_b[:],
                                               t0[:], OP.mult, OP.add)
                cnp.append(pc)
            ctxq = []
            for kb in range(HB):
                q = cq_pool.tile([P, HALF], F16, tag="cq")
                src = cn_keep[kb][:] if kb < NHC else cnp[kb - NHC][:]
                t1 = stream2.tile([P, HALF], F16, tag="cq16")
                if kb % 2 == 0:
                    nc.scalar.activation(t1[:], src, AF.Identity,
                                         bias=mgb[:], scale=sctx_b[:])
                else:
                    nc.vector.tensor_scalar(t1[:], src, sctx_b[:], MG,
                                            OP.mult, OP.add)
                nc.vector.tensor_scalar(q[:], t1[:], MG, None,
                                        OP.subtract)
                ctxq.append(q)

            def dense_pass(tb, psum, kbs):
                for kb in kbs:
                    for sl in _chunks(4, 512):
                        nc.tensor.matmul(
                            psum[:, sl],
                            lhsT=ctxq[kb][:, tb * P:(tb + 1) * P],
                            rhs=wd_sb[kb][:, sl],
                            start=(kb == 0), stop=(kb == HB - 1))

            # local-head (kb<8) passes need only the locally-quantized ctx,
            # so they start while the partner halves are still being
            # combined/quantized; partner passes trail by two blocks.
            psums = {}
            for t in (0, 1):
                psums[t] = ps3.tile([P, H], F32, tag="ps", name="dps%d" % t)
                dense_pass(t, psums[t], range(NHC))
            for tb in range(NHC):
                psum = psums[tb]
                dense_pass(tb, psum, range(NHC, HB))
                nt = tb + 2
                if nt < NHC:
                    psums[nt] = ps3.tile([P, H], F32, tag="ps",
                                         name="dps%d" % nt)
                    dense_pass(nt, psums[nt], range(NHC))
                # residual + layernorm straight off the psum
                x_t = lnx_pool.tile([P, H], F32, tag="lnx")
                nc.sync.dma_start(x_t[:], xr[tb * P:(tb + 1) * P, :])
                y = ev3_pool.tile([P, H], F32, tag="lny")
                ysum = lns_pool.tile([P, 1], F32, tag="ysum")
                if d_bias_zero:
                    nc.vector.scalar_tensor_tensor(
                        y[:], psum[:], alphad_b[:], x_t[:],
                        OP.mult, OP.add, accum_out=ysum[:])
                else:
                    t = sq_pool.tile([P, H], F32, tag="lnt")
                    nc.vector.scalar_tensor_tensor(
                        t[:], psum[:], alphad_b[:], bdb[:], OP.mult, OP.add)
                    nc.vector.scalar_tensor_tensor(
                        y[:], x_t[:], 1.0, t[:], OP.mult, OP.add,
                        accum_out=ysum[:])
                nmu = lns_pool.tile([P, 1], F32, tag="nmu")
                nc.vector.tensor_scalar(nmu[:], ysum[:], -1.0 / H, None,
                                        OP.mult)
                sq = sq_pool.tile([P, H], F32, tag="lnsq")
                sqs = lns_pool.tile([P, 1], F32, tag="sqs")
                nc.scalar.activation(sq[:], y[:], AF.Square,
                                     bias=nmu[:], scale=1.0,
                                     accum_out=sqs[:])
                v1 = lns_pool.tile([P, 1], F32, tag="v1")
                nc.vector.tensor_scalar(v1[:], sqs[:], 1.0 / H, LN_EPS,
                                        OP.mult, OP.add)
                v2 = lns_pool.tile([P, 1], F32, tag="v2")
                nc.vector.reciprocal(v2[:], v1[:])
                rstd = lns_pool.tile([P, 1], F32, tag="rstd")
                nc.scalar.activation(rstd[:], v2[:], AF.Sqrt)
                nmr = lns_pool.tile([P, 1], F32, tag="nmr")
                nc.vector.tensor_tensor(nmr[:], nmu[:], rstd[:], OP.mult)

                yn = ev3_pool.tile([P, H], F32, tag="lnyn")
                nc.scalar.activation(yn[:], y[:], AF.Identity,
                                     bias=nmr[:], scale=rstd[:])
                if not ln_trivial:
                    nc.vector.tensor_tensor(yn[:], yn[:], lnwb[:], OP.mult)
                    nc.vector.tensor_tensor(yn[:], yn[:], lnbb[:], OP.add)
                nc.sync.dma_start(out[tb * P:(tb + 1) * P, :], yn[:])
        cn_es.close()
        wd_es.close()


# ======================= host side =======================================

def make_in_maps(hidden_states, attention_mask, W_qkv, b_qkv, W_dense,
                 b_dense, ln_w, ln_b):
    x = np.asarray(hidden_states, dtype=np.float32)
    mask = np.asarray(attention_mask, dtype=np.float32)
    Wq = np.asarray(W_qkv, dtype=np.float32)
    bq = np.asarray(b_qkv, dtype=np.float32)
    Wd = np.asarray(W_dense, dtype=np.float32)
    bd = np.asarray(b_dense, dtype=np.float32)
    lw = np.asarray(ln_w, dtype=np.float32)
    lb = np.asarray(ln_b, dtype=np.float32)

    WdT = Wd.T.astype(np.float16)  # [h, out]
    in_maps = []
    for c in range(NCORES):
        b, g = c // 2, c % 2
        # per-core token permutation: own half first
        tord = np.r_[g * 1024:(g + 1) * 1024,
                     (1 - g) * 1024:(1 - g) * 1024 + 1024]
        sl = slice(g * 1024, (g + 1) * 1024)
        wq_g = Wq[sl, :]
        wk_g = Wq[2048 + g * 1024:2048 + (g + 1) * 1024, :]
        wv_g = Wq[4096 + g * 1024:4096 + (g + 1) * 1024, :]
        bq_g = bq[sl]
        bk_g = bq[2048 + g * 1024:2048 + (g + 1) * 1024]
        bv_g = bq[4096 + g * 1024:4096 + (g + 1) * 1024]
        W2 = np.concatenate([wq_g, wk_g], axis=0).T  # [h, out]
        wq_tiled = np.ascontiguousarray(
            W2.reshape(16, P, 16, P).transpose(2, 1, 0, 3)
            .reshape(16, P, H).astype(np.float16))
        # W_dense^T rows: own head half first, then partner's
        wdt_c = np.ascontiguousarray(np.concatenate(
            [WdT[g * 1024:(g + 1) * 1024, :],
             WdT[(1 - g) * 1024:(1 - g) * 1024 + 1024, :]], axis=0))
        in_maps.append({
            "xt": np.ascontiguousarray(x[b].T[:, tord].astype(np.float16)),
            "xr": np.ascontiguousarray(x[b, g * 1024:(g + 1) * 1024, :]),
            "wqkt": wq_tiled,
            "wvt": np.ascontiguousarray(wv_g.T.astype(np.float16)),
            "bqk": np.ascontiguousarray(
                np.concatenate([bq_g, bk_g]).reshape(16, P).T),
            "bv": bv_g.reshape(1, 1024).copy(),
            "wdt": wdt_c,
            "bdh": bd.reshape(1, H).copy(),
            "maskt": np.ascontiguousarray(
                mask[b, 0, 0, tord].reshape(HB, P).T),
            "csel": np.eye(16, dtype=np.float32)[2 + c].reshape(1, 16).copy(),
            "pmsk": np.array([[float(g), float(1 - g)]], dtype=np.float32),
            "lnw": lw.reshape(1, H).copy(),
            "lnb": lb.reshape(1, H).copy(),
        })
    return in_maps


def build_flags(attention_mask, b_qkv, b_dense, ln_w, ln_b):
    return (
        bool(np.any(np.asarray(attention_mask) != 0.0)),
        bool(np.all(np.asarray(b_qkv)[:4096] == 0.0)),
        bool(np.all(np.asarray(b_qkv)[4096:] == 0.0)),
        bool(np.all(np.asarray(b_dense) == 0.0)),
        bool(np.all(np.asarray(ln_w) == 1.0) and np.all(np.asarray(ln_b) == 0.0)),
    )


def assemble_output(results):
    full = np.empty((B, S, H), dtype=np.float32)
    for c in range(NCORES):
        b, g = c // 2, c % 2
        full[b, g * 1024:(g + 1) * 1024, :] = results[c]["out"]
    return full


_CACHE = {}


def _get_program(flags):
    if flags not in _CACHE:
        _CACHE[flags] = build_program(*flags)
    return _CACHE[flags]


def _ensure_ntff_hook():
    """Provide antenv.axon_hooks (missing in this image) so trace=True can
    capture NTFF profiles through the axon PJRT plugin."""
    import types

    try:
        import antenv.axon_hooks  # noqa: F401
        return
    except ImportError:
        pass
    try:
        import antenv
    except ImportError:
        return
    mod = types.ModuleType("antenv.axon_hooks")
    holder = {"h": None}
    mod.set_axon_ntff_profile_hook = lambda h: holder.__setitem__("h", h)
    mod.get_axon_ntff_profile_hook = lambda: holder["h"]
    sys.modules["antenv.axon_hooks"] = mod
    antenv.axon_hooks = mod
    try:
        if "/root/.axon_site" not in sys.path:
            sys.path.insert(0, "/root/.axon_site")
        from trn_agent_boot.trn_boot import _ntff_profile_via_ctypes
        h = _ntff_profile_via_ctypes("/opt/axon/libaxon_pjrt.so")
        if h is not None:
            mod.set_axon_ntff_profile_hook(h)
    except Exception:
        pass


def kernel(hidden_states, attention_mask, W_qkv, b_qkv, W_dense, b_dense,
           ln_w, ln_b, trace=False):
    from concourse.bass_utils import run_bass_kernel_spmd

    flags = build_flags(attention_mask, b_qkv, b_dense, ln_w, ln_b)
    nc = _get_program(flags)
    in_maps = make_in_maps(hidden_states, attention_mask, W_qkv, b_qkv,
                           W_dense, b_dense, ln_w, ln_b)
    if trace:
        _ensure_ntff_hook()
        try:
            res = run_bass_kernel_spmd(nc, in_maps,
                                       core_ids=list(range(NCORES)),
                                       trace=True)
        except Exception as e:
            print("trace run failed (%s); retrying untraced" % e)
            res = run_bass_kernel_spmd(nc, in_maps,
                                       core_ids=list(range(NCORES)),
                                       trace=False)
    else:
        res = run_bass_kernel_spmd(nc, in_maps, core_ids=list(range(NCORES)),
                                   trace=False)
    out = assemble_output(res.results)
    kernel.last_result = res
    return out
